# revision 43
# baseline (speedup 1.0000x reference)
"""TRN2 Bass kernel for nn_DSSMEmbed (vq_codebook), v4.

Strategy (8 NeuronCores, data-parallel over batch, 256 imgs/core):
  - Fold emb conv into c1 (both towers): cascade-exact composite 5x5
    operators applied to windowed one-hot inputs (exact in bf16), with
    x-border restrictions folded into per-group ops, y-border handled by
    two correction matmuls, and a 15th "ones" channel carrying the
    emb-bias validity term.
  - Tower2 (feeds VQ argmax, needs ~fp32 scores): all matmuls bf16 with
    hi/lo weight/activation splitting (2-pass stage A with exact one-hot
    moving operand; 3-pass c2 and fused scores).
  - Tower2 linear fused with VQ scoring: M2 = lw2.T @ znT on host;
    scores = X3 @ M2 + b_l2 @ znT.
  - v4: partition-major DRAM layouts -> few big DMAs (>=1KB/partition
    runs); 4 issuing queues; M2/lw1 streamed in 1MB chunks; incremental
    c2 window builds (subtile deps); per-core local zn gather +
    transpose then AllGather of the transposed zT slice; embed1
    normalization folded into fin activation scale.
"""
import sys

sys.path.insert(0, "/opt/trn_rl_repo")

import numpy as np
import concourse.bass as bass
import concourse.bacc as bacc
import concourse.mybir as mybir
import concourse.tile as tile
from concourse.bass_utils import run_bass_kernel_spmd

F32 = mybir.dt.float32
BF16 = mybir.dt.bfloat16
U32 = mybir.dt.uint32
AF = mybir.ActivationFunctionType

NCORES = 8
B = 2048
BL = B // NCORES          # 256 imgs per core
H = W = 16
DICT, SE, CE, ESZ, NZ = 14, 8, 16, 512, 512
NCH = DICT + 1            # +1 ones channel for emb-bias border term
EPS = 1e-4
YB = H * BL               # free dim (y, img) = 4096

# ---------------------------------------------------------------------------
# host-side preprocessing
# ---------------------------------------------------------------------------


def _hi(x):
    import ml_dtypes
    return np.asarray(x, np.float32).astype(ml_dtypes.bfloat16)


def _lo(x):
    import ml_dtypes
    x = np.asarray(x, np.float32)
    return (x - x.astype(ml_dtypes.bfloat16).astype(np.float32)).astype(
        ml_dtypes.bfloat16)


def make_win_onehot(nat):
    """nat: (NCH, H, W, Bloc) -> (4, 128, H, Bloc) int8 full-y windows.

    Group g serves x-outs 4g..4g+3 via window x' = 4g-2+w, w in 0..7;
    rows w*NCH + c (120 used). Out-of-image x' rows stay 0.
    """
    out = np.zeros((4, 128, H, nat.shape[-1]), dtype=np.int8)
    for g in range(4):
        for w in range(8):
            xs = 4 * g - 2 + w
            if 0 <= xs < W:
                out[g, w * NCH:(w + 1) * NCH] = nat[:, :, xs, :]
    return out


def stageA_ops(c1w, embfold, embb):
    """Cascade-exact composite operators for c1(emb(x)).

    Returns (5+2, 4, 128, 64): dy-tap operators (U in 0..4) followed by
    corr_bot (output row y=0) and corr_top (y=15) correction operators.
    """
    c1w = np.asarray(c1w, np.float32)
    embb = np.asarray(embb, np.float32)
    kb = np.einsum("ocuv,c->ouv", c1w, embb)  # (16, 3, 3)

    def bvalid(g, xr):
        x = 4 * g + xr
        lob = 1 if x == 0 else 0
        hib = 1 if x == W - 1 else 2
        return lob, hib

    ops = np.zeros((7, 4, 128, 64), np.float32)
    for g in range(4):
        for xr in range(4):
            lob, hib = bvalid(g, xr)
            for a in range(3):
                for u in range(3):
                    U = a + u
                    for b in range(lob, hib + 1):
                        for v in range(3):
                            w = b + v + xr
                            if not (0 <= w < 8):
                                continue
                            ops[U, g, w * NCH:w * NCH + DICT,
                                xr * 16:(xr + 1) * 16] += np.einsum(
                                "oc,cd->do", c1w[:, :, a, b],
                                embfold[:, :, u, v])
            # ones channel: centered 3x3 kernel kb, no b-restriction
            for a in range(3):
                for b in range(3):
                    ops[a + 1, g, (b + 1 + xr) * NCH + DICT,
                        xr * 16:(xr + 1) * 16] += kb[:, a, b]
            # y-border corrections
            for b in range(lob, hib + 1):
                for v in range(3):
                    w = b + v + xr
                    if not (0 <= w < 8):
                        continue
                    ops[5, g, w * NCH:w * NCH + DICT,
                        xr * 16:(xr + 1) * 16] -= np.einsum(
                        "oc,cd->do", c1w[:, :, 0, b], embfold[:, :, 2, v])
                    ops[6, g, w * NCH:w * NCH + DICT,
                        xr * 16:(xr + 1) * 16] -= np.einsum(
                        "oc,cd->do", c1w[:, :, 2, b], embfold[:, :, 0, v])
    return ops


def op_c2(wc, dy):
    """3x3 c2 operator for one dy: (128, 128) (g-independent).

    wc: (32, 16, 3, 3). lhsT[w*16 + ci, xr*32 + co] = wc[co, ci, dy, w-xr]
    for 0 <= w - xr <= 2 (x' = 4g-1+w, w in 0..5; x = 4g+xr).
    """
    blk = np.zeros((128, 128), np.float32)
    for w in range(6):
        for xr in range(4):
            dx = w - xr
            if 0 <= dx <= 2:
                blk[w * 16:(w + 1) * 16, xr * 32:(xr + 1) * 32] = \
                    wc[:, :, dy, dx].T
    return blk


def reorder_lin(lw):
    """(ESZ, 8192) -> (64, 128, ESZ): k-tile t=(g, y), row r = xr*32+ch,
    source index ch*256 + y*16 + (4g + xr)."""
    lw = np.asarray(lw, np.float32).reshape(-1, 32, H, W)  # (E, ch, y, x)
    E = lw.shape[0]
    lw = lw.transpose(3, 2, 1, 0).reshape(4, 4, H, 32, E)  # (g, xr, y, ch, E)
    lw = lw.transpose(0, 2, 1, 3, 4).reshape(4, H, 128, E)
    return np.ascontiguousarray(lw.reshape(64, 128, E))


def pmaj(a):
    """(..., 128, w) -> (128, prod(...)*w) partition-major contiguous."""
    a = np.asarray(a)
    sh = a.shape
    lead = int(np.prod(sh[:-2])) if len(sh) > 2 else 1
    a = a.reshape(lead, 128, sh[-1])
    return np.ascontiguousarray(a.transpose(1, 0, 2).reshape(128, -1))


def host_prep(inputs):
    s = np.asarray(inputs["s"])
    sp = np.asarray(inputs["s_prime"])
    se_w = np.asarray(inputs["state_embed"], dtype=np.float32)
    norms = np.sqrt((se_w * se_w).sum(1, keepdims=True))
    table = se_w / np.maximum(norms, 1.0)
    embfold = np.einsum("oikl,di->odkl",
                        np.asarray(inputs["conv_embed_w"], np.float32), table)

    ar = np.arange(DICT)
    oh_s = (ar[:, None, None, None] == s.transpose(1, 2, 0)[None]).astype(
        np.int8)
    oh_sp = (ar[:, None, None, None] == sp.transpose(1, 2, 0)[None]).astype(
        np.int8)
    ones_row = np.ones((1, H, W, B), np.int8)
    nat_s = np.concatenate([oh_s, ones_row], axis=0)
    nat_d = np.concatenate([(oh_sp - oh_s), np.zeros_like(ones_row)], axis=0)

    opA_t1 = stageA_ops(inputs["p1c1_w"], embfold, inputs["conv_embed_b"])
    opA_t2 = stageA_ops(inputs["p2c1_w"], embfold, inputs["conv_embed_b"])
    opC_t1 = np.stack([op_c2(np.asarray(inputs["p1c2_w"], np.float32), dy)
                       for dy in range(3)])                      # (3,128,128)
    opC_t2 = np.stack([op_c2(np.asarray(inputs["p2c2_w"], np.float32), dy)
                       for dy in range(3)])

    zv = np.asarray(inputs["z_vectors"], np.float32)
    zn = zv / np.sqrt((zv * zv).sum(1, keepdims=True))
    M2 = np.asarray(inputs["p2l_w"], np.float32).T @ zn.T  # (8192, NZ)
    M2re = reorder_lin(M2.T)                               # (64, 128, NZ)
    lw1re = reorder_lin(inputs["p1l_w"])                   # (64, 128, ESZ)
    brow = (np.asarray(inputs["p2l_b"], np.float32) @ zn.T).reshape(1, NZ)

    def conv_bias(bvec, c_out):
        reps = 128 // c_out
        return np.ascontiguousarray(
            np.tile(np.asarray(bvec, np.float32), reps)[:, None])

    shared = {
        "opA_t1": pmaj(_hi(opA_t1)),            # (128, 28*64)
        "opA_t2h": pmaj(_hi(opA_t2)),
        "opA_t2l": pmaj(_lo(opA_t2)),
        "opC_t1": pmaj(_hi(opC_t1)),            # (128, 3*128)
        "opC_t2h": pmaj(_hi(opC_t2)),
        "opC_t2l": pmaj(_lo(opC_t2)),
        "M2h": pmaj(_hi(M2re)),                 # (128, 64*512)
        "M2l": pmaj(_lo(M2re)),
        "lw1": pmaj(_hi(lw1re)),                # (128, 64*512)
        "znb": _hi(zn),                         # (512, 512)
        "bias4": np.ascontiguousarray(np.concatenate(
            [conv_bias(inputs["p1c1_b"], 16), conv_bias(inputs["p2c1_b"], 16),
             conv_bias(inputs["p1c2_b"], 32), conv_bias(inputs["p2c2_b"], 32)],
            axis=1)),
        "b_l1": np.ascontiguousarray(
            np.asarray(inputs["p1l_b"], np.float32).reshape(1, ESZ)),
        "browh": np.ascontiguousarray(_hi(brow)),
        "browl": np.ascontiguousarray(_lo(brow)),
        "identb": _hi(np.eye(128, dtype=np.float32)),
        "zeros32": _hi(np.zeros((32, H * BL), np.float32)),
    }
    esc = float(np.exp(np.asarray(inputs["scale"], np.float32).reshape(-1)[0]))

    percore = []
    for c in range(NCORES):
        sl = slice(c * BL, (c + 1) * BL)
        ws = make_win_onehot(nat_s[..., sl])
        wd = make_win_onehot(nat_d[..., sl])
        percore.append({
            "ohs": pmaj(_hi(ws.reshape(4, 128, H * BL))),  # (128, 4*H*BL)
            "ohd": pmaj(_hi(wd.reshape(4, 128, H * BL))),
        })
    return shared, percore, esc


# ---------------------------------------------------------------------------
# device program
# ---------------------------------------------------------------------------


def _clip_dy(y0, ny, dy):
    s = max(y0, -dy)
    e = min(y0 + ny, H - dy)
    if s >= e:
        return None
    return (s - y0) * BL, (e - s) * BL, s + dy


def build_program(esc):
    from contextlib import ExitStack
    nc = bacc.Bacc("TRN2", target_bir_lowering=False, debug=False,
                   num_devices=NCORES)

    def din(name, shape, dt):
        return nc.dram_tensor(name, list(shape), dt, kind="ExternalInput").ap()

    ohs_d = din("ohs", (128, 4 * H * BL), BF16)
    ohd_d = din("ohd", (128, 4 * H * BL), BF16)
    opA_t1_d = din("opA_t1", (128, 28 * 64), BF16)
    opA_t2h_d = din("opA_t2h", (128, 28 * 64), BF16)
    opA_t2l_d = din("opA_t2l", (128, 28 * 64), BF16)
    opC_t1_d = din("opC_t1", (128, 3 * 128), BF16)
    opC_t2h_d = din("opC_t2h", (128, 3 * 128), BF16)
    opC_t2l_d = din("opC_t2l", (128, 3 * 128), BF16)
    M2h_d = din("M2h", (128, 64 * NZ), BF16)
    M2l_d = din("M2l", (128, 64 * NZ), BF16)
    lw1_d = din("lw1", (128, 64 * ESZ), BF16)
    znb_d = din("znb", (NZ, ESZ), BF16)
    bias4_d = din("bias4", (128, 4), F32)
    b_l1_d = din("b_l1", (1, ESZ), F32)
    browh_d = din("browh", (1, NZ), BF16)
    browl_d = din("browl", (1, NZ), BF16)
    identb_d = din("identb", (128, 128), BF16)
    zeros32_d = din("zeros32", (32, H * BL), BF16)

    out_d = nc.dram_tensor("out", [BL, B], F32, kind="ExternalOutput").ap()
    # local transposed z-rows slice and its allgather
    zT_loc_d = nc.dram_tensor("zT_loc", [128, 4, BL], BF16).ap()
    zT_g_d = nc.dram_tensor("zT_g", [NCORES, 128, 4, BL], BF16,
                            addr_space="Shared").ap()

    # DMA queue helpers: 4 issuing engines
    with tile.TileContext(nc) as tc, ExitStack() as ES:
        cst = ES.enter_context(tc.tile_pool(name="cst", bufs=1))
        npool = ES.enter_context(tc.tile_pool(name="nat", bufs=1))
        epool = ES.enter_context(tc.tile_pool(name="emb", bufs=1))

        QS = [nc.sync, nc.scalar, nc.gpsimd, nc.sync]

        # ---- t2 windows first (cold-start critical): 4 g x (rows 0:4,
        # 4:16) pieces, one queue per g ----
        ohd_v = ohd_d.rearrange("p (g y i) -> p g y i", g=4, y=H)
        ohs_v = ohs_d.rearrange("p (g y i) -> p g y i", g=4, y=H)
        wins_t2 = [npool.tile([128, H, BL], BF16, tag=f"C{g}",
                              name=f"winC{g}") for g in range(4)]
        for g in range(4):
            QS[g].dma_start(wins_t2[g][:, 0:4, :], ohd_v[:, g, 0:4, :])

        # ---- batched op loads (one DMA per set) ----
        def load_opA(d, pfx, eng):
            t = cst.tile([128, 28 * 64], BF16, tag=pfx, name=pfx)
            eng.dma_start(t[:], d[:])
            return [[t[:, (dy * 4 + g) * 64:(dy * 4 + g + 1) * 64]
                     for g in range(4)] for dy in range(7)]

        def load_opC(d, pfx, eng):
            t = cst.tile([128, 3 * 128], BF16, tag=pfx, name=pfx)
            eng.dma_start(t[:], d[:])
            return [t[:, dy * 128:(dy + 1) * 128] for dy in range(3)]

        opsA_t2h = load_opA(opA_t2h_d, "a2h", nc.sync)
        opsA_t2l = load_opA(opA_t2l_d, "a2l", nc.scalar)
        for g in range(4):
            QS[g].dma_start(wins_t2[g][:, 4:10, :], ohd_v[:, g, 4:10, :])
        for g in range(4):
            QS[g].dma_start(wins_t2[g][:, 10:H, :], ohd_v[:, g, 10:H, :])
        opsC_t2h = load_opC(opC_t2h_d, "c2h", nc.scalar)
        opsC_t2l = load_opC(opC_t2l_d, "c2l", nc.scalar)
        opsC_t1 = load_opC(opC_t1_d, "c1", nc.gpsimd)
        opsA_t1 = load_opA(opA_t1_d, "a1", nc.gpsimd)

        ones_k = cst.tile([1, 128], F32, tag="ones_k", name="ones_k")
        nc.vector.memset(ones_k[:], 1.0)
        onesb = cst.tile([128, 1], BF16, tag="onesb", name="onesb")
        nc.vector.memset(onesb[:], 1.0)
        ones_kb = cst.tile([1, 128], BF16, tag="ones_kb", name="ones_kb")
        nc.vector.memset(ones_kb[:], 1.0)
        bias4 = cst.tile([128, 4], F32, tag="bias4", name="bias4")
        nc.scalar.dma_start(bias4[:], bias4_d[:])
        bias_sb = {nm: bias4[:, i:i + 1] for i, nm in enumerate(
            ["b_c1t1", "b_c1t2", "b_c2t1", "b_c2t2"])}
        bl_sb = {}
        for nm, d, dt_ in [("b_l1", b_l1_d, F32), ("browh", browh_d, BF16),
                           ("browl", browl_d, BF16)]:
            t = cst.tile([1, ESZ], dt_, tag=f"{nm}r", name=f"{nm}r")
            nc.sync.dma_start(t[:], d[:])
            bl_sb[nm] = t
        identb = cst.tile([128, 128], BF16, tag="identb", name="identb")
        nc.scalar.dma_start(identb[:], identb_d[:])

        # ---------------- stage A: composite 5x5 from one-hot windows ------
        zv32 = zeros32_d.rearrange("p (y i) -> p y i", y=H)

        def alloc_win_c2(wp, tag, g, engs, memset=True):
            w = wp.tile([128, H, BL], BF16, tag=tag, name=tag)
            if memset:
                engs[0].dma_start(w[96:128, :, :], zv32[:])
                if g == 0:
                    engs[1 % len(engs)].dma_start(w[0:32, :, :], zv32[:])
                if g == 3:
                    engs[1 % len(engs)].dma_start(w[64:96, :, :], zv32[:])
            return w

        def win_c2_pieces(g):
            """Return [(dst_row0, nrows, src_tile_idx, src_row0)] for g."""
            x0 = 4 * g - 1
            xs_s, xs_e = max(0, x0), min(W, x0 + 6)
            pieces = []
            a = xs_s
            while a < xs_e:
                bb = min(xs_e, 8 if a < 8 else 16)
                pieces.append(((a - x0) * 16, (bb - a) * 16, a // 8,
                               (a % 8) * 16))
                a = bb
            return pieces

        def fill_win_c2(w, g, srcs, yg, engs):
            y0, y1 = 2 * yg, 2 * yg + 2
            for j, (d0, nr, ch, r0) in enumerate(win_c2_pieces(g)):
                src = srcs[ch].rearrange("p (y i) -> p y i", y=H)
                engs[j % len(engs)].dma_start(
                    w[d0:d0 + nr, y0:y1, :], src[r0:r0 + nr, y0:y1, :])

        def stageA(wins, op_list, bias, out_tags, hilo, cb=None):
            outs = [[npool.tile([128, YB], BF16, tag=tg, name=tg)
                     for tg in tgs] for tgs in out_tags]
            with tc.tile_pool(name=f"At{out_tags[0][0]}", bufs=2) as tp, \
                 tc.tile_pool(name=f"Ap{out_tags[0][0]}", bufs=2,
                              space="PSUM") as pp:
                for yg in range(8):
                    y0 = 2 * yg
                    ps = [pp.tile([128, 2 * BL], F32, tag=f"p{i}",
                                  name=f"p{i}") for i in range(2)]
                    mm = []
                    for dy in (0, -1, 1, -2, 2):
                        cl = _clip_dy(y0, 2, dy)
                        if cl is None:
                            continue
                        n0, N, ysrc = cl
                        for ops in op_list:
                            for g in range(4):
                                mm.append((ops[dy + 2][g], g, n0, N, ysrc,
                                           N // BL))
                    if yg == 0:
                        for ops in op_list:
                            for g in range(4):
                                mm.append((ops[5][g], g, 0, BL, 0, 1))
                    if yg == 7:
                        for ops in op_list:
                            for g in range(4):
                                mm.append((ops[6][g], g, BL, BL, 15, 1))
                    first_g, last_g = {}, {}
                    for i, (op, g, n0, N, ysrc, nys) in enumerate(mm):
                        first_g.setdefault(g, i)
                        last_g[g] = i
                    for i, (op, g, n0, N, ysrc, nys) in enumerate(mm):
                        nc.tensor.matmul(
                            ps[g // 2][64 * (g % 2):64 * (g % 2) + 64,
                                       n0:n0 + N],
                            op,
                            wins[g][:, ysrc:ysrc + nys, :],
                            start=(i == first_g[g]), stop=(i == last_g[g]),
                            tile_position=(0, 64 * (g % 2)))
                    sl = slice(y0 * BL, (y0 + 2) * BL)
                    for i in range(2):
                        if hilo:
                            tmp = tp.tile([128, 2 * BL], F32, tag="t",
                                          name="tmpa")
                            nc.scalar.activation(tmp[:], ps[i][:],
                                                 AF.Relu, bias=bias)
                            nc.vector.tensor_copy(outs[0][i][:, sl], tmp[:])
                            nc.vector.tensor_sub(outs[1][i][:, sl],
                                                 tmp[:], outs[0][i][:, sl])
                        elif i % 2 == 0:
                            nc.scalar.activation(outs[0][i][:, sl],
                                                 ps[i][:], AF.Relu,
                                                 bias=bias)
                        else:
                            nc.vector.tensor_scalar(
                                outs[0][i][:, sl], ps[i][:], bias, 0.0,
                                mybir.AluOpType.add, mybir.AluOpType.max)
                    if cb is not None:
                        cb(yg, outs)
            return outs

        # ---------------- c2 conv (3x3, window-6, M=128, g-outer) ----------
        def c2_conv(winHs, srcL, op_list, bias, out_tags, hilo, winL0=None):
            outs = [[npool.tile([128, YB], BF16, tag=tg, name=tg)
                     for tg in tgs] for tgs in out_tags]
            with tc.tile_pool(name=f"Ct{out_tags[0][0]}", bufs=2) as tp, \
                 tc.tile_pool(name=f"Cp{out_tags[0][0]}", bufs=3,
                              space="PSUM") as pp:
                for g in range(4):
                    passes = [(op_list[0], winHs[g])]
                    if hilo:
                        if g == 0 and winL0 is not None:
                            winL = winL0
                        else:
                            winL = alloc_win_c2(wcl, "wl", g,
                                                [nc.sync, nc.scalar])
                            for j, (d0, nr, ch, r0) in enumerate(
                                    win_c2_pieces(g)):
                                src = srcL[ch].rearrange(
                                    "p (y i) -> p y i", y=H)
                                for half, eng in ((0, nc.sync),
                                                  (1, nc.scalar)):
                                    eng.dma_start(
                                        winL[d0:d0 + nr,
                                             8 * half:8 * half + 8, :],
                                        src[r0:r0 + nr,
                                            8 * half:8 * half + 8, :])
                        passes += [(op_list[1], winHs[g]),
                                   (op_list[0], winL)]
                    for yg in range(8):
                        y0 = 2 * yg
                        ps = pp.tile([128, 2 * BL], F32, tag="p", name="p")
                        mm = []
                        for dy in (0, -1, 1):
                            cl = _clip_dy(y0, 2, dy)
                            if cl is None:
                                continue
                            n0, N, ysrc = cl
                            for (ops, win) in passes:
                                mm.append((ops[dy + 1], win, n0, N, ysrc,
                                           N // BL))
                        for i, (op, win, n0, N, ysrc, nys) in enumerate(mm):
                            nc.tensor.matmul(
                                ps[:, n0:n0 + N],
                                op, win[:, ysrc:ysrc + nys, :],
                                start=(i == 0), stop=(i == len(mm) - 1))
                        sl = slice(y0 * BL, (y0 + 2) * BL)
                        if hilo:
                            tmp = tp.tile([128, 2 * BL], F32, tag="t",
                                          name="tmpc")
                            nc.scalar.activation(tmp[:], ps[:], AF.Relu,
                                                 bias=bias)
                            nc.vector.tensor_copy(outs[0][g][:, sl], tmp[:])
                            nc.vector.tensor_sub(outs[1][g][:, sl], tmp[:],
                                                 outs[0][g][:, sl])
                        elif yg % 2 == 0:
                            nc.scalar.activation(outs[0][g][:, sl], ps[:],
                                                 AF.Relu, bias=bias)
                        else:
                            nc.vector.tensor_scalar(
                                outs[0][g][:, sl], ps[:], bias, 0.0,
                                mybir.AluOpType.add, mybir.AluOpType.max)
            return outs

        # ================== tower 2 ==================
        wp2 = ES.enter_context(tc.tile_pool(name="wc2", bufs=1))
        winH_t2 = [alloc_win_c2(wp2, f"wh{g}", g, [nc.sync, nc.scalar])
                   for g in range(4)]
        from contextlib import ExitStack as _ES2
        wcl_scope = _ES2()
        wcl = wcl_scope.enter_context(tc.tile_pool(name="wcl", bufs=2))
        winL0_t2 = alloc_win_c2(wcl, "wl", 0, [nc.sync, nc.scalar])

        def cb_t2A(yg, outs):
            for g in range(4):
                fill_win_c2(winH_t2[g], g, outs[0], yg,
                            [nc.sync, nc.scalar])
            fill_win_c2(winL0_t2, 0, outs[1], yg, [nc.scalar, nc.sync])

        with nc.named_scope("t2A"):
            X2h, X2l = stageA(wins_t2, [opsA_t2h, opsA_t2l],
                              bias_sb["b_c1t2"],
                              [["A0", "A1"], ["B0", "B1"]], hilo=True,
                              cb=cb_t2A)

        with nc.named_scope("t2c2"):
            X3h, X3l = c2_conv(winH_t2, X2l, [opsC_t2h, opsC_t2l],
                               bias_sb["b_c2t2"],
                               [["C0", "C1", "C2", "C3"],
                                ["D0", "D1", "D2", "D3"]], hilo=True,
                               winL0=winL0_t2)
        wcl_scope.close()

        # prefetch t1 stage-A windows into the freed X2 slots (sync and
        # scalar queues idle through t2sc)
        wins_t1 = []
        for g, tg in enumerate(["A0", "A1", "B0", "B1"]):
            wt = npool.tile([128, H, BL], BF16, tag=tg, name=f"w1{tg}")
            [nc.sync, nc.scalar][g % 2].dma_start(wt[:], ohs_v[:, g, :, :])
            wins_t1.append(wt)

        # -------- fused scores: X3 @ M2 + brow; argmax -> local codes ------
        M2h_v = M2h_d.rearrange("p (k n) -> p k n", n=NZ)
        M2l_v = M2l_d.rearrange("p (k n) -> p k n", n=NZ)
        KC = 8                     # k-tiles per chunk
        zrp = ES.enter_context(tc.tile_pool(name="zrp", bufs=1))
        with nc.named_scope("t2sc"):
            with tc.tile_pool(name="m2p", bufs=2) as mwp, \
                 tc.tile_pool(name="scp", bufs=1) as scp, \
                 tc.tile_pool(name="spp", bufs=1, space="PSUM") as spp:
                sps = [spp.tile([128, NZ], F32, tag=f"s{m}", name=f"s{m}")
                       for m in range(2)]
                for k0 in range(0, 64, KC):
                    mh = mwp.tile([128, KC * NZ], BF16, tag="mh", name="mh")
                    ml = mwp.tile([128, KC * NZ], BF16, tag="ml", name="ml")
                    nc.gpsimd.dma_start(mh[:], M2h_v[:, k0:k0 + KC, :])
                    nc.gpsimd.dma_start(ml[:], M2l_v[:, k0:k0 + KC, :])
                    for kk in range(KC):
                        k = k0 + kk
                        g, y = k // 16, k % 16
                        mhs = mh[:, kk * NZ:(kk + 1) * NZ]
                        mls = ml[:, kk * NZ:(kk + 1) * NZ]
                        for m in range(2):
                            c0 = y * BL + 128 * m
                            nc.tensor.matmul(sps[m][:],
                                             X3h[g][:, c0:c0 + 128], mhs,
                                             start=(k == 0), stop=False)
                            nc.tensor.matmul(sps[m][:],
                                             X3l[g][:, c0:c0 + 128], mhs,
                                             start=False, stop=False)
                            nc.tensor.matmul(sps[m][:],
                                             X3h[g][:, c0:c0 + 128], mls,
                                             start=False, stop=False)
                for m in range(2):
                    nc.tensor.matmul(sps[m][:], ones_kb[:],
                                     bl_sb["browh"][:, 0:NZ], start=False,
                                     stop=False)
                    nc.tensor.matmul(sps[m][:], ones_kb[:],
                                     bl_sb["browl"][:, 0:NZ], start=False,
                                     stop=True)
                idxs = []
                for m in range(2):
                    mx = scp.tile([128, 8], F32, tag=f"mx{m}", name=f"mx{m}")
                    nc.vector.max(mx[:], sps[m][:])
                    ix = zrp.tile([128, 8], U32, tag=f"ix{m}", name=f"ix{m}")
                    nc.vector.max_index(ix[:], mx[:], sps[m][:])
                    idxs.append(ix)

        # ---- gather own z-rows, transpose, allgather (emitted mid-t1) ----
        def emit_gather_ag():
            zrs = []
            for m in range(2):
                zr = zrp.tile([128, ESZ], BF16, tag=f"zr{m}", name=f"zr{m}")
                nc.gpsimd.indirect_dma_start(
                    out=zr[:], out_offset=None, in_=znb_d[:],
                    in_offset=bass.IndirectOffsetOnAxis(
                        ap=idxs[m][:, 0:1], axis=0))
                zrs.append(zr)
            zT_loc = zrp.tile([128, 4, BL], BF16, tag="zTl", name="zTl")
            with tc.tile_pool(name="tpb", bufs=1, space="PSUM") as tpb:
                for m in range(2):
                    for e in range(4):
                        tb = tpb.tile([128, 128], BF16, tag="tb", name="tb")
                        nc.tensor.transpose(tb[:],
                                            zrs[m][:, 128 * e:128 * e + 128],
                                            identb[:])
                        nc.vector.tensor_copy(
                            zT_loc[:, e, 128 * m:128 * m + 128], tb[:])
            nc.sync.dma_start(zT_loc_d[:], zT_loc[:])
            nc.gpsimd.collective_compute(
                "AllGather", mybir.AluOpType.bypass,
                replica_groups=[list(range(NCORES))],
                ins=[zT_loc_d[:]], outs=[zT_g_d[:]])

        # ================== tower 1 (bf16) ==================
        lwp = ES.enter_context(tc.tile_pool(name="lwp", bufs=3))
        winH_t1 = [alloc_win_c2(wp2, f"wh{g}", g, [nc.sync], memset=False)
                   for g in range(4)]

        def cb_t1A(yg, outs):
            for g in range(4):
                fill_win_c2(winH_t1[g], g, outs[0], yg,
                            [nc.sync, nc.gpsimd])

        with nc.named_scope("t1A"):
            (Y2,) = stageA(wins_t1, [opsA_t1], bias_sb["b_c1t1"],
                           [["D0", "D1"]], hilo=False, cb=cb_t1A)
        emit_gather_ag()
        with nc.named_scope("t1c2"):
            (Y3,) = c2_conv(winH_t1, None, [opsC_t1], bias_sb["b_c2t1"],
                            [["C0", "C1", "C2", "C3"]], hilo=False)

        # ---------------- t1 linear -> embT1 (img, ESZ) --------------------
        lw1_v = lw1_d.rearrange("p (k n) -> p k n", n=ESZ)
        with nc.named_scope("t1lin"):
            embT1 = [epool.tile([128, ESZ], F32, tag=f"e1T{m}",
                                name=f"e1T{m}") for m in range(2)]
            with tc.tile_pool(name="lpp", bufs=1, space="PSUM") as lpp:
                ps = [lpp.tile([128, ESZ], F32, tag=f"p{m}", name=f"p{m}")
                      for m in range(2)]
                KCL = 8
                for k0 in range(0, 64, KCL):
                    lwt = lwp.tile([128, KCL * ESZ], BF16, tag="lw",
                                   name="lw")
                    [nc.sync, nc.scalar][(k0 // KCL) % 2].dma_start(
                        lwt[:], lw1_v[:, k0:k0 + KCL, :])
                    for kk in range(KCL):
                        k = k0 + kk
                        g, y = k // 16, k % 16
                        lws = lwt[:, kk * ESZ:(kk + 1) * ESZ]
                        for m in range(2):
                            c0 = y * BL + 128 * m
                            nc.tensor.matmul(ps[m][:], Y3[g][:, c0:c0 + 128],
                                             lws, start=(k == 0), stop=False)
                for m in range(2):
                    nc.tensor.matmul(ps[m][:], ones_k[:], bl_sb["b_l1"][:],
                                     start=False, stop=True)
                e1bf = [epool.tile([128, ESZ], BF16, tag=f"ebf{m}",
                                   name=f"ebf{m}") for m in range(2)]
                for m in range(2):
                    nc.scalar.activation(embT1[m][:], ps[m][:], AF.Identity)
                    nc.scalar.activation(e1bf[m][:], ps[m][:], AF.Identity)

            # rnt[m] = exp(scale) / (|e1| + eps)  (applied at fin output)
            n2s = []
            with tc.tile_pool(name="nrm", bufs=2) as nrp:
                for m in range(2):
                    sq = nrp.tile([128, ESZ], F32, tag=f"sq{m}",
                                  name=f"sq{m}")
                    nc.vector.tensor_mul(sq[:], embT1[m][:], embT1[m][:])
                    n2 = epool.tile([128, 1], F32, tag=f"n2{m}",
                                    name=f"n2{m}")
                    nc.vector.tensor_reduce(n2[:], sq[:],
                                            mybir.AxisListType.X,
                                            mybir.AluOpType.add)
                    nc.scalar.sqrt(n2[:], n2[:])
                    nc.vector.tensor_scalar_add(n2[:], n2[:], EPS)
                    nc.vector.reciprocal(n2[:], n2[:])
                    nc.vector.tensor_scalar_mul(n2[:], n2[:], esc)
                    n2s.append(n2)

            # transpose embT1 (unnormalized) to e1b (E, img) bf16
            e1b = [epool.tile([128, BL], BF16, tag=f"e1b{e}", name=f"e1b{e}")
                   for e in range(4)]
            with tc.tile_pool(name="tpp", bufs=4, space="PSUM") as tpp:
                for m in range(2):
                    for e in range(4):
                        tp = tpp.tile([128, 128], BF16, tag="tp", name="tp")
                        nc.tensor.transpose(tp[:],
                                            e1bf[m][:, 128 * e:128 * e + 128],
                                            identb[:])
                        nc.vector.tensor_copy(e1b[e][:, 128 * m:128 * m + 128],
                                              tp[:])

        # ---- load gathered zT (E, B): per e-block strided from zT_g ------
        ztp = ES.enter_context(tc.tile_pool(name="ztp", bufs=1))
        zT = [ztp.tile([128, B], BF16, tag=f"zT{e}", name=f"zT{e}")
              for e in range(4)]
        with tc.tile_wait_until(0.5):
            for e, qe in enumerate([nc.sync, nc.scalar, nc.scalar, nc.sync]):
                qe.dma_start(
                    zT[e].rearrange("p (c i) -> p c i", c=NCORES),
                    zT_g_d[:, :, e, :].rearrange("c p i -> p c i"))

        # ---------------- final: out = rnt * (e1 @ zT) ---------------------
        # e-outer with one PSUM tile per (m, n): fin matmuls for e start
        # as soon as e1b[e] is transposed, overlapping the transposes
        with nc.named_scope("fin"):
            with tc.tile_pool(name="fob", bufs=2) as fob, \
                 tc.tile_pool(name="fpp", bufs=1, space="PSUM") as fpp:
                fps = [[fpp.tile([128, 512], F32, tag=f"fp{m}{n}",
                                 name=f"fp{m}{n}") for n in range(4)]
                       for m in range(2)]
                for m in range(2):
                    for e in range(4):
                        for n in range(4):
                            nc.tensor.matmul(
                                fps[m][n][:],
                                e1b[e][:, 128 * m:128 * m + 128],
                                zT[e][:, 512 * n:512 * n + 512],
                                start=(e == 0), stop=(e == 3))
                    for n in range(4):
                        ob = fob.tile([128, 512], F32, tag="ob", name="ob")
                        if n % 2 == 0:
                            nc.scalar.activation(ob[:], fps[m][n][:],
                                                 AF.Identity,
                                                 scale=n2s[m][:])
                        else:
                            nc.vector.tensor_scalar_mul(ob[:], fps[m][n][:],
                                                        n2s[m][:])
                        (nc.sync if n % 2 == 0 else nc.scalar).dma_start(
                            out_d[128 * m:128 * m + 128,
                                  512 * n:512 * n + 512], ob[:])

    nc.compile()
    return nc


def make_in_maps(shared, percore):
    maps = []
    for pc in percore:
        m = dict(shared)
        m.update(pc)
        maps.append(m)
    return maps


def _run(inputs, trace=False):
    dsf = np.asarray(inputs.get("downscale_factor", 1)).reshape(-1)
    dsf = int(dsf[0]) if dsf.size else 1
    assert dsf == 1, f"only downscale_factor=1 supported, got {dsf}"
    shared, percore, esc = host_prep(inputs)
    nc = build_program(esc)
    maps = make_in_maps(shared, percore)
    res = run_bass_kernel_spmd(nc, maps, list(range(NCORES)), trace=trace)
    out = np.concatenate(
        [res.results[c]["out"] for c in range(NCORES)], axis=0)
    return np.ascontiguousarray(out, np.float32), res


def kernel(**inputs):
    out, _ = _run(inputs, trace=False)
    return out


def run_for_test(inputs, trace=False):
    return _run(inputs, trace=trace)


# revision 44
# speedup vs baseline: 1.0126x; 1.0126x over previous
"""TRN2 Bass kernel for nn_DSSMEmbed (vq_codebook), v4.

Strategy (8 NeuronCores, data-parallel over batch, 256 imgs/core):
  - Fold emb conv into c1 (both towers): cascade-exact composite 5x5
    operators applied to windowed one-hot inputs (exact in bf16), with
    x-border restrictions folded into per-group ops, y-border handled by
    two correction matmuls, and a 15th "ones" channel carrying the
    emb-bias validity term.
  - Tower2 (feeds VQ argmax, needs ~fp32 scores): all matmuls bf16 with
    hi/lo weight/activation splitting (2-pass stage A with exact one-hot
    moving operand; 3-pass c2 and fused scores).
  - Tower2 linear fused with VQ scoring: M2 = lw2.T @ znT on host;
    scores = X3 @ M2 + b_l2 @ znT.
  - v4: partition-major DRAM layouts -> few big DMAs (>=1KB/partition
    runs); 4 issuing queues; M2/lw1 streamed in 1MB chunks; incremental
    c2 window builds (subtile deps); per-core local zn gather +
    transpose then AllGather of the transposed zT slice; embed1
    normalization folded into fin activation scale.
"""
import sys

sys.path.insert(0, "/opt/trn_rl_repo")

import numpy as np
import concourse.bass as bass
import concourse.bacc as bacc
import concourse.mybir as mybir
import concourse.tile as tile
from concourse.bass_utils import run_bass_kernel_spmd

F32 = mybir.dt.float32
BF16 = mybir.dt.bfloat16
U32 = mybir.dt.uint32
AF = mybir.ActivationFunctionType

NCORES = 8
B = 2048
BL = B // NCORES          # 256 imgs per core
H = W = 16
DICT, SE, CE, ESZ, NZ = 14, 8, 16, 512, 512
NCH = DICT + 1            # +1 ones channel for emb-bias border term
EPS = 1e-4
YB = H * BL               # free dim (y, img) = 4096

# ---------------------------------------------------------------------------
# host-side preprocessing
# ---------------------------------------------------------------------------


def _hi(x):
    import ml_dtypes
    return np.asarray(x, np.float32).astype(ml_dtypes.bfloat16)


def _lo(x):
    import ml_dtypes
    x = np.asarray(x, np.float32)
    return (x - x.astype(ml_dtypes.bfloat16).astype(np.float32)).astype(
        ml_dtypes.bfloat16)


def make_win_onehot(nat):
    """nat: (NCH, H, W, Bloc) -> (4, 128, H, Bloc) int8 full-y windows.

    Group g serves x-outs 4g..4g+3 via window x' = 4g-2+w, w in 0..7;
    rows w*NCH + c (120 used). Out-of-image x' rows stay 0.
    """
    out = np.zeros((4, 128, H, nat.shape[-1]), dtype=np.int8)
    for g in range(4):
        for w in range(8):
            xs = 4 * g - 2 + w
            if 0 <= xs < W:
                out[g, w * NCH:(w + 1) * NCH] = nat[:, :, xs, :]
    return out


def stageA_ops(c1w, embfold, embb):
    """Cascade-exact composite operators for c1(emb(x)).

    Returns (5+2, 4, 128, 64): dy-tap operators (U in 0..4) followed by
    corr_bot (output row y=0) and corr_top (y=15) correction operators.
    """
    c1w = np.asarray(c1w, np.float32)
    embb = np.asarray(embb, np.float32)
    kb = np.einsum("ocuv,c->ouv", c1w, embb)  # (16, 3, 3)

    def bvalid(g, xr):
        x = 4 * g + xr
        lob = 1 if x == 0 else 0
        hib = 1 if x == W - 1 else 2
        return lob, hib

    ops = np.zeros((7, 4, 128, 64), np.float32)
    for g in range(4):
        for xr in range(4):
            lob, hib = bvalid(g, xr)
            for a in range(3):
                for u in range(3):
                    U = a + u
                    for b in range(lob, hib + 1):
                        for v in range(3):
                            w = b + v + xr
                            if not (0 <= w < 8):
                                continue
                            ops[U, g, w * NCH:w * NCH + DICT,
                                xr * 16:(xr + 1) * 16] += np.einsum(
                                "oc,cd->do", c1w[:, :, a, b],
                                embfold[:, :, u, v])
            # ones channel: centered 3x3 kernel kb, no b-restriction
            for a in range(3):
                for b in range(3):
                    ops[a + 1, g, (b + 1 + xr) * NCH + DICT,
                        xr * 16:(xr + 1) * 16] += kb[:, a, b]
            # y-border corrections
            for b in range(lob, hib + 1):
                for v in range(3):
                    w = b + v + xr
                    if not (0 <= w < 8):
                        continue
                    ops[5, g, w * NCH:w * NCH + DICT,
                        xr * 16:(xr + 1) * 16] -= np.einsum(
                        "oc,cd->do", c1w[:, :, 0, b], embfold[:, :, 2, v])
                    ops[6, g, w * NCH:w * NCH + DICT,
                        xr * 16:(xr + 1) * 16] -= np.einsum(
                        "oc,cd->do", c1w[:, :, 2, b], embfold[:, :, 0, v])
    return ops


def op_c2(wc, dy):
    """3x3 c2 operator for one dy: (128, 128) (g-independent).

    wc: (32, 16, 3, 3). lhsT[w*16 + ci, xr*32 + co] = wc[co, ci, dy, w-xr]
    for 0 <= w - xr <= 2 (x' = 4g-1+w, w in 0..5; x = 4g+xr).
    """
    blk = np.zeros((128, 128), np.float32)
    for w in range(6):
        for xr in range(4):
            dx = w - xr
            if 0 <= dx <= 2:
                blk[w * 16:(w + 1) * 16, xr * 32:(xr + 1) * 32] = \
                    wc[:, :, dy, dx].T
    return blk


def reorder_lin(lw):
    """(ESZ, 8192) -> (64, 128, ESZ): k-tile t=(g, y), row r = xr*32+ch,
    source index ch*256 + y*16 + (4g + xr)."""
    lw = np.asarray(lw, np.float32).reshape(-1, 32, H, W)  # (E, ch, y, x)
    E = lw.shape[0]
    lw = lw.transpose(3, 2, 1, 0).reshape(4, 4, H, 32, E)  # (g, xr, y, ch, E)
    lw = lw.transpose(0, 2, 1, 3, 4).reshape(4, H, 128, E)
    return np.ascontiguousarray(lw.reshape(64, 128, E))


def pmaj(a):
    """(..., 128, w) -> (128, prod(...)*w) partition-major contiguous."""
    a = np.asarray(a)
    sh = a.shape
    lead = int(np.prod(sh[:-2])) if len(sh) > 2 else 1
    a = a.reshape(lead, 128, sh[-1])
    return np.ascontiguousarray(a.transpose(1, 0, 2).reshape(128, -1))


def host_prep(inputs):
    s = np.asarray(inputs["s"])
    sp = np.asarray(inputs["s_prime"])
    se_w = np.asarray(inputs["state_embed"], dtype=np.float32)
    norms = np.sqrt((se_w * se_w).sum(1, keepdims=True))
    table = se_w / np.maximum(norms, 1.0)
    embfold = np.einsum("oikl,di->odkl",
                        np.asarray(inputs["conv_embed_w"], np.float32), table)

    ar = np.arange(DICT)
    oh_s = (ar[:, None, None, None] == s.transpose(1, 2, 0)[None]).astype(
        np.int8)
    oh_sp = (ar[:, None, None, None] == sp.transpose(1, 2, 0)[None]).astype(
        np.int8)
    ones_row = np.ones((1, H, W, B), np.int8)
    nat_s = np.concatenate([oh_s, ones_row], axis=0)
    nat_d = np.concatenate([(oh_sp - oh_s), np.zeros_like(ones_row)], axis=0)

    opA_t1 = stageA_ops(inputs["p1c1_w"], embfold, inputs["conv_embed_b"])
    opA_t2 = stageA_ops(inputs["p2c1_w"], embfold, inputs["conv_embed_b"])
    opC_t1 = np.stack([op_c2(np.asarray(inputs["p1c2_w"], np.float32), dy)
                       for dy in range(3)])                      # (3,128,128)
    opC_t2 = np.stack([op_c2(np.asarray(inputs["p2c2_w"], np.float32), dy)
                       for dy in range(3)])

    zv = np.asarray(inputs["z_vectors"], np.float32)
    zn = zv / np.sqrt((zv * zv).sum(1, keepdims=True))
    M2 = np.asarray(inputs["p2l_w"], np.float32).T @ zn.T  # (8192, NZ)
    M2re = reorder_lin(M2.T)                               # (64, 128, NZ)
    lw1re = reorder_lin(inputs["p1l_w"])                   # (64, 128, ESZ)
    brow = (np.asarray(inputs["p2l_b"], np.float32) @ zn.T).reshape(1, NZ)

    def conv_bias(bvec, c_out):
        reps = 128 // c_out
        return np.ascontiguousarray(
            np.tile(np.asarray(bvec, np.float32), reps)[:, None])

    shared = {
        "opA_t1": pmaj(_hi(opA_t1)),            # (128, 28*64)
        "opA_t2h": pmaj(_hi(opA_t2)),
        "opA_t2l": pmaj(_lo(opA_t2)),
        "opC_t1": pmaj(_hi(opC_t1)),            # (128, 3*128)
        "opC_t2h": pmaj(_hi(opC_t2)),
        "opC_t2l": pmaj(_lo(opC_t2)),
        "M2h": pmaj(_hi(M2re)),                 # (128, 64*512)
        "M2l": pmaj(_lo(M2re)),
        "lw1": pmaj(_hi(lw1re)),                # (128, 64*512)
        "znb": _hi(zn),                         # (512, 512)
        "bias4": np.ascontiguousarray(np.concatenate(
            [conv_bias(inputs["p1c1_b"], 16), conv_bias(inputs["p2c1_b"], 16),
             conv_bias(inputs["p1c2_b"], 32), conv_bias(inputs["p2c2_b"], 32)],
            axis=1)),
        "b_l1": np.ascontiguousarray(
            np.asarray(inputs["p1l_b"], np.float32).reshape(1, ESZ)),
        "browh": np.ascontiguousarray(_hi(brow)),
        "browl": np.ascontiguousarray(_lo(brow)),
        "identb": _hi(np.eye(128, dtype=np.float32)),
        "zeros32": _hi(np.zeros((32, H * BL), np.float32)),
    }
    esc = float(np.exp(np.asarray(inputs["scale"], np.float32).reshape(-1)[0]))

    percore = []
    for c in range(NCORES):
        sl = slice(c * BL, (c + 1) * BL)
        ws = make_win_onehot(nat_s[..., sl])
        wd = make_win_onehot(nat_d[..., sl])
        percore.append({
            "ohs": pmaj(_hi(ws.reshape(4, 128, H * BL))),  # (128, 4*H*BL)
            "ohd": pmaj(_hi(wd.reshape(4, 128, H * BL))),
        })
    return shared, percore, esc


# ---------------------------------------------------------------------------
# device program
# ---------------------------------------------------------------------------


def _clip_dy(y0, ny, dy):
    s = max(y0, -dy)
    e = min(y0 + ny, H - dy)
    if s >= e:
        return None
    return (s - y0) * BL, (e - s) * BL, s + dy


def build_program(esc):
    from contextlib import ExitStack
    nc = bacc.Bacc("TRN2", target_bir_lowering=False, debug=False,
                   num_devices=NCORES)

    def din(name, shape, dt):
        return nc.dram_tensor(name, list(shape), dt, kind="ExternalInput").ap()

    ohs_d = din("ohs", (128, 4 * H * BL), BF16)
    ohd_d = din("ohd", (128, 4 * H * BL), BF16)
    opA_t1_d = din("opA_t1", (128, 28 * 64), BF16)
    opA_t2h_d = din("opA_t2h", (128, 28 * 64), BF16)
    opA_t2l_d = din("opA_t2l", (128, 28 * 64), BF16)
    opC_t1_d = din("opC_t1", (128, 3 * 128), BF16)
    opC_t2h_d = din("opC_t2h", (128, 3 * 128), BF16)
    opC_t2l_d = din("opC_t2l", (128, 3 * 128), BF16)
    M2h_d = din("M2h", (128, 64 * NZ), BF16)
    M2l_d = din("M2l", (128, 64 * NZ), BF16)
    lw1_d = din("lw1", (128, 64 * ESZ), BF16)
    znb_d = din("znb", (NZ, ESZ), BF16)
    bias4_d = din("bias4", (128, 4), F32)
    b_l1_d = din("b_l1", (1, ESZ), F32)
    browh_d = din("browh", (1, NZ), BF16)
    browl_d = din("browl", (1, NZ), BF16)
    identb_d = din("identb", (128, 128), BF16)
    zeros32_d = din("zeros32", (32, H * BL), BF16)

    out_d = nc.dram_tensor("out", [BL, B], F32, kind="ExternalOutput").ap()
    # local transposed z-rows slice and its allgather
    zT_loc_d = nc.dram_tensor("zT_loc", [128, 4, BL], BF16).ap()
    zT_g_d = nc.dram_tensor("zT_g", [NCORES, 128, 4, BL], BF16,
                            addr_space="Shared").ap()

    # DMA queue helpers: 4 issuing engines
    with tile.TileContext(nc) as tc, ExitStack() as ES:
        cst = ES.enter_context(tc.tile_pool(name="cst", bufs=1))
        npool = ES.enter_context(tc.tile_pool(name="nat", bufs=1))
        epool = ES.enter_context(tc.tile_pool(name="emb", bufs=1))

        QS = [nc.sync, nc.scalar, nc.gpsimd, nc.sync]

        # ---- t2 windows first (cold-start critical): 4 g x (rows 0:4,
        # 4:16) pieces, one queue per g ----
        ohd_v = ohd_d.rearrange("p (g y i) -> p g y i", g=4, y=H)
        ohs_v = ohs_d.rearrange("p (g y i) -> p g y i", g=4, y=H)
        wins_t2 = [npool.tile([128, H, BL], BF16, tag=f"C{g}",
                              name=f"winC{g}") for g in range(4)]
        for g in range(4):
            QS[g].dma_start(wins_t2[g][:, 0:4, :], ohd_v[:, g, 0:4, :])

        # ---- batched op loads (one DMA per set) ----
        def load_opA(d, pfx, eng):
            t = cst.tile([128, 28 * 64], BF16, tag=pfx, name=pfx)
            eng.dma_start(t[:], d[:])
            return [[t[:, (dy * 4 + g) * 64:(dy * 4 + g + 1) * 64]
                     for g in range(4)] for dy in range(7)]

        def load_opC(d, pfx, eng):
            t = cst.tile([128, 3 * 128], BF16, tag=pfx, name=pfx)
            eng.dma_start(t[:], d[:])
            return [t[:, dy * 128:(dy + 1) * 128] for dy in range(3)]

        opsA_t2h = load_opA(opA_t2h_d, "a2h", nc.sync)
        opsA_t2l = load_opA(opA_t2l_d, "a2l", nc.scalar)
        for g in range(4):
            QS[g].dma_start(wins_t2[g][:, 4:10, :], ohd_v[:, g, 4:10, :])
        for g in range(4):
            QS[g].dma_start(wins_t2[g][:, 10:H, :], ohd_v[:, g, 10:H, :])
        opsC_t2h = load_opC(opC_t2h_d, "c2h", nc.scalar)
        opsC_t2l = load_opC(opC_t2l_d, "c2l", nc.scalar)
        opsC_t1 = load_opC(opC_t1_d, "c1", nc.gpsimd)
        opsA_t1 = load_opA(opA_t1_d, "a1", nc.gpsimd)

        ones_k = cst.tile([1, 128], F32, tag="ones_k", name="ones_k")
        nc.vector.memset(ones_k[:], 1.0)
        onesb = cst.tile([128, 1], BF16, tag="onesb", name="onesb")
        nc.vector.memset(onesb[:], 1.0)
        ones_kb = cst.tile([1, 128], BF16, tag="ones_kb", name="ones_kb")
        nc.vector.memset(ones_kb[:], 1.0)
        bias4 = cst.tile([128, 4], F32, tag="bias4", name="bias4")
        nc.scalar.dma_start(bias4[:], bias4_d[:])
        bias_sb = {nm: bias4[:, i:i + 1] for i, nm in enumerate(
            ["b_c1t1", "b_c1t2", "b_c2t1", "b_c2t2"])}
        bl_sb = {}
        for nm, d, dt_ in [("b_l1", b_l1_d, F32), ("browh", browh_d, BF16),
                           ("browl", browl_d, BF16)]:
            t = cst.tile([1, ESZ], dt_, tag=f"{nm}r", name=f"{nm}r")
            nc.sync.dma_start(t[:], d[:])
            bl_sb[nm] = t
        identb = cst.tile([128, 128], BF16, tag="identb", name="identb")
        nc.scalar.dma_start(identb[:], identb_d[:])

        # ---------------- stage A: composite 5x5 from one-hot windows ------
        zv32 = zeros32_d.rearrange("p (y i) -> p y i", y=H)

        def alloc_win_c2(wp, tag, g, engs, memset=True):
            w = wp.tile([128, H, BL], BF16, tag=tag, name=tag)
            if memset:
                engs[0].dma_start(w[96:128, :, :], zv32[:])
                if g == 0:
                    engs[1 % len(engs)].dma_start(w[0:32, :, :], zv32[:])
                if g == 3:
                    engs[1 % len(engs)].dma_start(w[64:96, :, :], zv32[:])
            return w

        def win_c2_pieces(g):
            """Return [(dst_row0, nrows, src_tile_idx, src_row0)] for g."""
            x0 = 4 * g - 1
            xs_s, xs_e = max(0, x0), min(W, x0 + 6)
            pieces = []
            a = xs_s
            while a < xs_e:
                bb = min(xs_e, 8 if a < 8 else 16)
                pieces.append(((a - x0) * 16, (bb - a) * 16, a // 8,
                               (a % 8) * 16))
                a = bb
            return pieces

        def fill_win_c2(w, g, srcs, yg, engs):
            y0, y1 = 2 * yg, 2 * yg + 2
            for j, (d0, nr, ch, r0) in enumerate(win_c2_pieces(g)):
                src = srcs[ch].rearrange("p (y i) -> p y i", y=H)
                engs[j % len(engs)].dma_start(
                    w[d0:d0 + nr, y0:y1, :], src[r0:r0 + nr, y0:y1, :])

        def stageA(wins, op_list, bias, out_tags, hilo, cb=None):
            outs = [[npool.tile([128, YB], BF16, tag=tg, name=tg)
                     for tg in tgs] for tgs in out_tags]
            with tc.tile_pool(name=f"At{out_tags[0][0]}", bufs=2) as tp, \
                 tc.tile_pool(name=f"Ap{out_tags[0][0]}", bufs=2,
                              space="PSUM") as pp:
                for yg in range(8):
                    y0 = 2 * yg
                    ps = [pp.tile([128, 2 * BL], F32, tag=f"p{i}",
                                  name=f"p{i}") for i in range(2)]
                    mm = []
                    for dy in (0, -1, 1, -2, 2):
                        cl = _clip_dy(y0, 2, dy)
                        if cl is None:
                            continue
                        n0, N, ysrc = cl
                        for ops in op_list:
                            for g in range(4):
                                mm.append((ops[dy + 2][g], g, n0, N, ysrc,
                                           N // BL))
                    if yg == 0:
                        for ops in op_list:
                            for g in range(4):
                                mm.append((ops[5][g], g, 0, BL, 0, 1))
                    if yg == 7:
                        for ops in op_list:
                            for g in range(4):
                                mm.append((ops[6][g], g, BL, BL, 15, 1))
                    first_g, last_g = {}, {}
                    for i, (op, g, n0, N, ysrc, nys) in enumerate(mm):
                        first_g.setdefault(g, i)
                        last_g[g] = i
                    for i, (op, g, n0, N, ysrc, nys) in enumerate(mm):
                        nc.tensor.matmul(
                            ps[g // 2][64 * (g % 2):64 * (g % 2) + 64,
                                       n0:n0 + N],
                            op,
                            wins[g][:, ysrc:ysrc + nys, :],
                            start=(i == first_g[g]), stop=(i == last_g[g]),
                            tile_position=(0, 64 * (g % 2)))
                    sl = slice(y0 * BL, (y0 + 2) * BL)
                    for i in range(2):
                        if hilo:
                            tmp = tp.tile([128, 2 * BL], F32, tag="t",
                                          name="tmpa")
                            nc.scalar.activation(tmp[:], ps[i][:],
                                                 AF.Relu, bias=bias)
                            nc.vector.tensor_copy(outs[0][i][:, sl], tmp[:])
                            nc.vector.tensor_sub(outs[1][i][:, sl],
                                                 tmp[:], outs[0][i][:, sl])
                        elif i % 2 == 0:
                            nc.scalar.activation(outs[0][i][:, sl],
                                                 ps[i][:], AF.Relu,
                                                 bias=bias)
                        else:
                            nc.vector.tensor_scalar(
                                outs[0][i][:, sl], ps[i][:], bias, 0.0,
                                mybir.AluOpType.add, mybir.AluOpType.max)
                    if cb is not None:
                        cb(yg, outs)
            return outs

        # ---------------- c2 conv (3x3, window-6, M=128, g-outer) ----------
        def c2_conv(winHs, srcL, op_list, bias, out_tags, hilo, winL0=None):
            outs = [[npool.tile([128, YB], BF16, tag=tg, name=tg)
                     for tg in tgs] for tgs in out_tags]
            with tc.tile_pool(name=f"Ct{out_tags[0][0]}", bufs=2) as tp, \
                 tc.tile_pool(name=f"Cp{out_tags[0][0]}", bufs=3,
                              space="PSUM") as pp:
                for g in range(4):
                    passes = [(op_list[0], winHs[g])]
                    if hilo:
                        if g == 0 and winL0 is not None:
                            winL = winL0
                        else:
                            winL = alloc_win_c2(wcl, "wl", g,
                                                [nc.sync, nc.scalar])
                            for j, (d0, nr, ch, r0) in enumerate(
                                    win_c2_pieces(g)):
                                src = srcL[ch].rearrange(
                                    "p (y i) -> p y i", y=H)
                                for half, eng in ((0, nc.sync),
                                                  (1, nc.scalar)):
                                    eng.dma_start(
                                        winL[d0:d0 + nr,
                                             8 * half:8 * half + 8, :],
                                        src[r0:r0 + nr,
                                            8 * half:8 * half + 8, :])
                        passes += [(op_list[1], winHs[g]),
                                   (op_list[0], winL)]
                    for yg in range(8):
                        y0 = 2 * yg
                        ps = pp.tile([128, 2 * BL], F32, tag="p", name="p")
                        mm = []
                        for dy in (0, -1, 1):
                            cl = _clip_dy(y0, 2, dy)
                            if cl is None:
                                continue
                            n0, N, ysrc = cl
                            for (ops, win) in passes:
                                mm.append((ops[dy + 1], win, n0, N, ysrc,
                                           N // BL))
                        for i, (op, win, n0, N, ysrc, nys) in enumerate(mm):
                            nc.tensor.matmul(
                                ps[:, n0:n0 + N],
                                op, win[:, ysrc:ysrc + nys, :],
                                start=(i == 0), stop=(i == len(mm) - 1))
                        sl = slice(y0 * BL, (y0 + 2) * BL)
                        if hilo:
                            tmp = tp.tile([128, 2 * BL], F32, tag="t",
                                          name="tmpc")
                            nc.scalar.activation(tmp[:], ps[:], AF.Relu,
                                                 bias=bias)
                            nc.vector.tensor_copy(outs[0][g][:, sl], tmp[:])
                            nc.vector.tensor_sub(outs[1][g][:, sl], tmp[:],
                                                 outs[0][g][:, sl])
                        elif yg % 2 == 0:
                            nc.scalar.activation(outs[0][g][:, sl], ps[:],
                                                 AF.Relu, bias=bias)
                        else:
                            nc.vector.tensor_scalar(
                                outs[0][g][:, sl], ps[:], bias, 0.0,
                                mybir.AluOpType.add, mybir.AluOpType.max)
            return outs

        # ================== tower 2 ==================
        wp2 = ES.enter_context(tc.tile_pool(name="wc2", bufs=1))
        winH_t2 = [alloc_win_c2(wp2, f"wh{g}", g, [nc.sync, nc.scalar])
                   for g in range(4)]
        from contextlib import ExitStack as _ES2
        wcl_scope = _ES2()
        wcl = wcl_scope.enter_context(tc.tile_pool(name="wcl", bufs=2))
        winL0_t2 = alloc_win_c2(wcl, "wl", 0, [nc.sync, nc.scalar])

        def cb_t2A(yg, outs):
            for g in range(4):
                fill_win_c2(winH_t2[g], g, outs[0], yg,
                            [nc.sync, nc.scalar])
            fill_win_c2(winL0_t2, 0, outs[1], yg, [nc.gpsimd])

        with nc.named_scope("t2A"):
            X2h, X2l = stageA(wins_t2, [opsA_t2h, opsA_t2l],
                              bias_sb["b_c1t2"],
                              [["A0", "A1"], ["B0", "B1"]], hilo=True,
                              cb=cb_t2A)

        with nc.named_scope("t2c2"):
            X3h, X3l = c2_conv(winH_t2, X2l, [opsC_t2h, opsC_t2l],
                               bias_sb["b_c2t2"],
                               [["C0", "C1", "C2", "C3"],
                                ["D0", "D1", "D2", "D3"]], hilo=True,
                               winL0=winL0_t2)
        wcl_scope.close()

        # prefetch t1 stage-A windows into the freed X2 slots (sync and
        # scalar queues idle through t2sc)
        wins_t1 = []
        for g, tg in enumerate(["A0", "A1", "B0", "B1"]):
            wt = npool.tile([128, H, BL], BF16, tag=tg, name=f"w1{tg}")
            [nc.sync, nc.scalar][g % 2].dma_start(wt[:], ohs_v[:, g, :, :])
            wins_t1.append(wt)

        # -------- fused scores: X3 @ M2 + brow; argmax -> local codes ------
        M2h_v = M2h_d.rearrange("p (k n) -> p k n", n=NZ)
        M2l_v = M2l_d.rearrange("p (k n) -> p k n", n=NZ)
        KC = 8                     # k-tiles per chunk
        zrp = ES.enter_context(tc.tile_pool(name="zrp", bufs=1))
        with nc.named_scope("t2sc"):
            with tc.tile_pool(name="m2p", bufs=2) as mwp, \
                 tc.tile_pool(name="scp", bufs=1) as scp, \
                 tc.tile_pool(name="spp", bufs=1, space="PSUM") as spp:
                sps = [spp.tile([128, NZ], F32, tag=f"s{m}", name=f"s{m}")
                       for m in range(2)]
                for k0 in range(0, 64, KC):
                    mh = mwp.tile([128, KC * NZ], BF16, tag="mh", name="mh")
                    ml = mwp.tile([128, KC * NZ], BF16, tag="ml", name="ml")
                    nc.gpsimd.dma_start(mh[:], M2h_v[:, k0:k0 + KC, :])
                    nc.gpsimd.dma_start(ml[:], M2l_v[:, k0:k0 + KC, :])
                    for kk in range(KC):
                        k = k0 + kk
                        g, y = k // 16, k % 16
                        mhs = mh[:, kk * NZ:(kk + 1) * NZ]
                        mls = ml[:, kk * NZ:(kk + 1) * NZ]
                        for m in range(2):
                            c0 = y * BL + 128 * m
                            nc.tensor.matmul(sps[m][:],
                                             X3h[g][:, c0:c0 + 128], mhs,
                                             start=(k == 0), stop=False)
                            nc.tensor.matmul(sps[m][:],
                                             X3l[g][:, c0:c0 + 128], mhs,
                                             start=False, stop=False)
                            nc.tensor.matmul(sps[m][:],
                                             X3h[g][:, c0:c0 + 128], mls,
                                             start=False, stop=False)
                for m in range(2):
                    nc.tensor.matmul(sps[m][:], ones_kb[:],
                                     bl_sb["browh"][:, 0:NZ], start=False,
                                     stop=False)
                    nc.tensor.matmul(sps[m][:], ones_kb[:],
                                     bl_sb["browl"][:, 0:NZ], start=False,
                                     stop=True)
                idxs = []
                for m in range(2):
                    mx = scp.tile([128, 8], F32, tag=f"mx{m}", name=f"mx{m}")
                    nc.vector.max(mx[:], sps[m][:])
                    ix = zrp.tile([128, 8], U32, tag=f"ix{m}", name=f"ix{m}")
                    nc.vector.max_index(ix[:], mx[:], sps[m][:])
                    idxs.append(ix)

        # ---- gather own z-rows, transpose, allgather (emitted mid-t1) ----
        def emit_gather_ag():
            zrs = []
            for m in range(2):
                zr = zrp.tile([128, ESZ], BF16, tag=f"zr{m}", name=f"zr{m}")
                nc.gpsimd.indirect_dma_start(
                    out=zr[:], out_offset=None, in_=znb_d[:],
                    in_offset=bass.IndirectOffsetOnAxis(
                        ap=idxs[m][:, 0:1], axis=0))
                zrs.append(zr)
            zT_loc = zrp.tile([128, 4, BL], BF16, tag="zTl", name="zTl")
            with tc.tile_pool(name="tpb", bufs=1, space="PSUM") as tpb:
                for m in range(2):
                    for e in range(4):
                        tb = tpb.tile([128, 128], BF16, tag="tb", name="tb")
                        nc.tensor.transpose(tb[:],
                                            zrs[m][:, 128 * e:128 * e + 128],
                                            identb[:])
                        nc.vector.tensor_copy(
                            zT_loc[:, e, 128 * m:128 * m + 128], tb[:])
            nc.sync.dma_start(zT_loc_d[:], zT_loc[:])
            nc.gpsimd.collective_compute(
                "AllGather", mybir.AluOpType.bypass,
                replica_groups=[list(range(NCORES))],
                ins=[zT_loc_d[:]], outs=[zT_g_d[:]])

        # ================== tower 1 (bf16) ==================
        lwp = ES.enter_context(tc.tile_pool(name="lwp", bufs=3))
        winH_t1 = [alloc_win_c2(wp2, f"wh{g}", g, [nc.sync], memset=False)
                   for g in range(4)]

        def cb_t1A(yg, outs):
            for g in range(4):
                fill_win_c2(winH_t1[g], g, outs[0], yg,
                            [nc.sync, nc.gpsimd])

        with nc.named_scope("t1A"):
            (Y2,) = stageA(wins_t1, [opsA_t1], bias_sb["b_c1t1"],
                           [["D0", "D1"]], hilo=False, cb=cb_t1A)
        emit_gather_ag()
        with nc.named_scope("t1c2"):
            (Y3,) = c2_conv(winH_t1, None, [opsC_t1], bias_sb["b_c2t1"],
                            [["C0", "C1", "C2", "C3"]], hilo=False)

        # ---------------- t1 linear -> embT1 (img, ESZ) --------------------
        lw1_v = lw1_d.rearrange("p (k n) -> p k n", n=ESZ)
        with nc.named_scope("t1lin"):
            embT1 = [epool.tile([128, ESZ], F32, tag=f"e1T{m}",
                                name=f"e1T{m}") for m in range(2)]
            with tc.tile_pool(name="lpp", bufs=1, space="PSUM") as lpp:
                ps = [lpp.tile([128, ESZ], F32, tag=f"p{m}", name=f"p{m}")
                      for m in range(2)]
                KCL = 8
                for k0 in range(0, 64, KCL):
                    lwt = lwp.tile([128, KCL * ESZ], BF16, tag="lw",
                                   name="lw")
                    [nc.sync, nc.scalar][(k0 // KCL) % 2].dma_start(
                        lwt[:], lw1_v[:, k0:k0 + KCL, :])
                    for kk in range(KCL):
                        k = k0 + kk
                        g, y = k // 16, k % 16
                        lws = lwt[:, kk * ESZ:(kk + 1) * ESZ]
                        for m in range(2):
                            c0 = y * BL + 128 * m
                            nc.tensor.matmul(ps[m][:], Y3[g][:, c0:c0 + 128],
                                             lws, start=(k == 0), stop=False)
                for m in range(2):
                    nc.tensor.matmul(ps[m][:], ones_k[:], bl_sb["b_l1"][:],
                                     start=False, stop=True)
                e1bf = [epool.tile([128, ESZ], BF16, tag=f"ebf{m}",
                                   name=f"ebf{m}") for m in range(2)]
                for m in range(2):
                    nc.scalar.activation(embT1[m][:], ps[m][:], AF.Identity)
                    nc.scalar.activation(e1bf[m][:], ps[m][:], AF.Identity)

            # rnt[m] = exp(scale) / (|e1| + eps)  (applied at fin output)
            n2s = []
            with tc.tile_pool(name="nrm", bufs=2) as nrp:
                for m in range(2):
                    sq = nrp.tile([128, ESZ], F32, tag=f"sq{m}",
                                  name=f"sq{m}")
                    nc.vector.tensor_mul(sq[:], embT1[m][:], embT1[m][:])
                    n2 = epool.tile([128, 1], F32, tag=f"n2{m}",
                                    name=f"n2{m}")
                    nc.vector.tensor_reduce(n2[:], sq[:],
                                            mybir.AxisListType.X,
                                            mybir.AluOpType.add)
                    nc.scalar.sqrt(n2[:], n2[:])
                    nc.vector.tensor_scalar_add(n2[:], n2[:], EPS)
                    nc.vector.reciprocal(n2[:], n2[:])
                    nc.vector.tensor_scalar_mul(n2[:], n2[:], esc)
                    n2s.append(n2)

            # transpose embT1 (unnormalized) to e1b (E, img) bf16
            e1b = [epool.tile([128, BL], BF16, tag=f"e1b{e}", name=f"e1b{e}")
                   for e in range(4)]
            with tc.tile_pool(name="tpp", bufs=4, space="PSUM") as tpp:
                for m in range(2):
                    for e in range(4):
                        tp = tpp.tile([128, 128], BF16, tag="tp", name="tp")
                        nc.tensor.transpose(tp[:],
                                            e1bf[m][:, 128 * e:128 * e + 128],
                                            identb[:])
                        nc.vector.tensor_copy(e1b[e][:, 128 * m:128 * m + 128],
                                              tp[:])

        # ---- load gathered zT (E, B): per e-block strided from zT_g ------
        ztp = ES.enter_context(tc.tile_pool(name="ztp", bufs=1))
        zT = [ztp.tile([128, B], BF16, tag=f"zT{e}", name=f"zT{e}")
              for e in range(4)]
        with tc.tile_wait_until(0.5):
            for e, qe in enumerate([nc.sync, nc.scalar, nc.scalar, nc.sync]):
                qe.dma_start(
                    zT[e].rearrange("p (c i) -> p c i", c=NCORES),
                    zT_g_d[:, :, e, :].rearrange("c p i -> p c i"))

        # ---------------- final: out = rnt * (e1 @ zT) ---------------------
        # e-outer with one PSUM tile per (m, n): fin matmuls for e start
        # as soon as e1b[e] is transposed, overlapping the transposes
        with nc.named_scope("fin"):
            with tc.tile_pool(name="fob", bufs=2) as fob, \
                 tc.tile_pool(name="fpp", bufs=1, space="PSUM") as fpp:
                fps = [[fpp.tile([128, 512], F32, tag=f"fp{m}{n}",
                                 name=f"fp{m}{n}") for n in range(4)]
                       for m in range(2)]
                for m in range(2):
                    for e in range(4):
                        for n in range(4):
                            nc.tensor.matmul(
                                fps[m][n][:],
                                e1b[e][:, 128 * m:128 * m + 128],
                                zT[e][:, 512 * n:512 * n + 512],
                                start=(e == 0), stop=(e == 3))
                    for n in range(4):
                        ob = fob.tile([128, 512], F32, tag="ob", name="ob")
                        if n % 2 == 0:
                            nc.scalar.activation(ob[:], fps[m][n][:],
                                                 AF.Identity,
                                                 scale=n2s[m][:])
                        else:
                            nc.vector.tensor_scalar_mul(ob[:], fps[m][n][:],
                                                        n2s[m][:])
                        (nc.sync if n % 2 == 0 else nc.scalar).dma_start(
                            out_d[128 * m:128 * m + 128,
                                  512 * n:512 * n + 512], ob[:])

    nc.compile()
    return nc


def make_in_maps(shared, percore):
    maps = []
    for pc in percore:
        m = dict(shared)
        m.update(pc)
        maps.append(m)
    return maps


def _run(inputs, trace=False):
    dsf = np.asarray(inputs.get("downscale_factor", 1)).reshape(-1)
    dsf = int(dsf[0]) if dsf.size else 1
    assert dsf == 1, f"only downscale_factor=1 supported, got {dsf}"
    shared, percore, esc = host_prep(inputs)
    nc = build_program(esc)
    maps = make_in_maps(shared, percore)
    res = run_bass_kernel_spmd(nc, maps, list(range(NCORES)), trace=trace)
    out = np.concatenate(
        [res.results[c]["out"] for c in range(NCORES)], axis=0)
    return np.ascontiguousarray(out, np.float32), res


def kernel(**inputs):
    out, _ = _run(inputs, trace=False)
    return out


def run_for_test(inputs, trace=False):
    return _run(inputs, trace=trace)
